# Initial kernel scaffold
#
"""DyConv (MoE-routed dynamic convolution) Trainium2 Bass kernel.

Data-parallel over batch: 32 samples -> 8 cores x 4 samples.
Per sample, fully on-device:
  gap  = mean(x, HW)                  (VectorE reduce, scale folded into w1)
  h    = relu(gap @ w1.T + b1)        (TensorE matmul K=Cin=128, ScalarE relu)
  l    = h @ w2.T + b2                (TensorE matmul K=17 w/ ones-row bias trick)
  r    = softmax(l / 30)              (VectorE max, ScalarE exp+sum, VectorE recip)
  kern = sum_e r[e] * convs[e]        (VectorE scalar_tensor_tensor FMAs)
  out  = conv2d(x, kern, pad=1)       (9 shifted matmuls accumulated in PSUM,
                                       Cin=128 partition contraction, Cout=2x128,
                                       7 row tiles of N=8*56=448, float32r)
"""

import os
from contextlib import ExitStack

import numpy as np

import concourse.bass as bass
import concourse.tile as tile
from concourse import mybir
from concourse.bass_utils import run_bass_kernel_spmd

F32 = mybir.dt.float32
F32R = mybir.dt.float32r

B, CIN, H, W = 32, 128, 56, 56
COUT, KS, E, R = 256, 3, 4, 16
NCORES = 8
BL = B // NCORES  # samples per core
TEMP = 30.0
HP, WP = H + 2, W + 2  # zero-padded image dims in SBUF
HWN = H * W  # 3136
ROWS_PER_TILE = 8
NTILES = H // ROWS_PER_TILE  # 7
NFREE = ROWS_PER_TILE * W  # 448 fp32 <= 512 (one PSUM bank)
TAPCO = KS * KS * COUT  # 2304, per-expert slice [tap, co]

# taps in kh-major order, matching the [ci, e, kh, kw, co] host layout
TAPS = [(dh, dw) for dh in (-1, 0, 1) for dw in (-1, 0, 1)]

# module-level knobs for test.py
TRACE = os.environ.get("DYCONV_TRACE", "0") == "1"
LAST_RESULTS = None
MM_DTYPE = F32R


def _build_program():
    nc = bass.Bass("TRN2", target_bir_lowering=False, debug=False)
    x_d = nc.dram_tensor("x", [BL, CIN, H, W], F32, kind="ExternalInput").ap()
    # host-prearranged: convs_r[ci, e, kh, kw, co] flattened to [128, E*9*COUT]
    convs_d = nc.dram_tensor("convs", [CIN, E * TAPCO], F32, kind="ExternalInput").ap()
    # w1.T / (H*W)  -> [CIN, R]
    w1t_d = nc.dram_tensor("w1t", [CIN, R], F32, kind="ExternalInput").ap()
    b1_d = nc.dram_tensor("b1", [R, 1], F32, kind="ExternalInput").ap()
    # [w2.T ; b2] -> [R+1, E]
    w2a_d = nc.dram_tensor("w2a", [R + 1, E], F32, kind="ExternalInput").ap()
    out_d = nc.dram_tensor("out", [BL, COUT, H, W], F32, kind="ExternalOutput").ap()

    with tile.TileContext(nc) as tc, ExitStack() as ctx:
        _emit(ctx, tc, x_d, convs_d, w1t_d, b1_d, w2a_d, out_d)
    return nc


def _emit(ctx, tc, x_d, convs_d, w1t_d, b1_d, w2a_d, out_d):
    nc = tc.nc

    const_pool = ctx.enter_context(tc.tile_pool(name="const", bufs=1))
    xp_pool = ctx.enter_context(tc.tile_pool(name="xpad", bufs=2))
    kern_pool = ctx.enter_context(tc.tile_pool(name="kern", bufs=2))
    small_pool = ctx.enter_context(tc.tile_pool(name="small", bufs=2))
    stage_pool = ctx.enter_context(tc.tile_pool(name="stage", bufs=2))
    psum_pool = ctx.enter_context(tc.tile_pool(name="psum", bufs=4, space="PSUM"))
    psum_r_pool = ctx.enter_context(tc.tile_pool(name="psum_r", bufs=2, space="PSUM"))

    # resident weights
    convs_sb = const_pool.tile([CIN, E * TAPCO], F32)
    nc.sync.dma_start(convs_sb[:], convs_d[:])
    w1t_sb = const_pool.tile([CIN, R], F32)
    nc.sync.dma_start(w1t_sb[:], w1t_d[:])
    b1_sb = const_pool.tile([R, 1], F32)
    nc.sync.dma_start(b1_sb[:], b1_d[:])
    w2a_sb = const_pool.tile([R + 1, E], F32)
    nc.sync.dma_start(w2a_sb[:], w2a_d[:])

    for b in range(BL):
        # ---- load x[b] into zero-padded SBUF image [128, 58, 58] ----
        xp = xp_pool.tile([CIN, HP, WP], F32)
        nc.gpsimd.memset(xp[:, 0, :], 0.0)                 # top border row
        nc.gpsimd.memset(xp[:, HP - 1, :], 0.0)            # bottom border row
        nc.gpsimd.memset(xp[:, 1 : HP - 1, 0:1], 0.0)      # left border col
        nc.gpsimd.memset(xp[:, 1 : HP - 1, WP - 1 : WP], 0.0)  # right border col
        nc.sync.dma_start(xp[:, 1 : H + 1, 1 : W + 1], x_d[b])

        # ---- router ----
        gap = small_pool.tile([CIN, 1], F32, tag="gap")
        nc.vector.reduce_sum(gap[:], xp[:, 1 : H + 1, 1 : W + 1], axis=mybir.AxisListType.XY)
        ph = psum_r_pool.tile([R, 1], F32, tag="ph")
        nc.tensor.matmul(ph[:], lhsT=w1t_sb[:], rhs=gap[:], start=True, stop=True)
        h_aug = small_pool.tile([R + 1, 1], F32, tag="haug")
        nc.vector.memset(h_aug[R : R + 1, :], 1.0)
        nc.scalar.activation(h_aug[0:R, :], ph[:], mybir.ActivationFunctionType.Relu,
                             bias=b1_sb[:], scale=1.0)
        pl = psum_r_pool.tile([1, E], F32, tag="pl")
        nc.tensor.matmul(pl[:], lhsT=h_aug[:], rhs=w2a_sb[:], start=True, stop=True)

        mx = small_pool.tile([1, 1], F32, tag="mx")
        nc.vector.reduce_max(mx[:], pl[:], axis=mybir.AxisListType.X)
        mxn = small_pool.tile([1, 1], F32, tag="mxn")
        nc.scalar.mul(mxn[:], mx[:], -1.0 / TEMP)
        ex = small_pool.tile([1, E], F32, tag="ex")
        ssum = small_pool.tile([1, 1], F32, tag="ssum")
        nc.scalar.activation(ex[:], pl[:], mybir.ActivationFunctionType.Exp,
                             bias=mxn[:], scale=1.0 / TEMP, accum_out=ssum[:])
        rec = small_pool.tile([1, 1], F32, tag="rec")
        nc.vector.reciprocal(rec[:], ssum[:])
        rt = small_pool.tile([1, E], F32, tag="rt")
        nc.vector.tensor_scalar_mul(rt[:], ex[:], rec[:])
        rb = small_pool.tile([CIN, E], F32, tag="rb")
        nc.gpsimd.partition_broadcast(rb[:], rt[0:1, :])

        # ---- mix expert kernels: kern[ci, tap, co] = sum_e r[e]*convs[ci,e,tap,co] ----
        kern = kern_pool.tile([CIN, TAPCO], F32)
        nc.vector.tensor_scalar_mul(kern[:], convs_sb[:, 0:TAPCO], rb[:, 0:1])
        for e in range(1, E):
            nc.vector.scalar_tensor_tensor(
                kern[:], convs_sb[:, e * TAPCO : (e + 1) * TAPCO], rb[:, e : e + 1],
                kern[:], op0=mybir.AluOpType.mult, op1=mybir.AluOpType.add)

        # ---- conv: out[co, h, w] = sum_tap sum_ci kern[ci,tap,co] * xshift[ci,h,w] ----
        for half in range(2):
            stage = stage_pool.tile([128, H, W], F32)
            for t in range(NTILES):
                ps = psum_pool.tile([128, ROWS_PER_TILE, W], F32)
                r0 = ROWS_PER_TILE * t
                for ki, (dh, dw) in enumerate(TAPS):
                    lhsT = kern[:, ki * COUT + half * 128 : ki * COUT + half * 128 + 128]
                    rhs = xp[:, 1 + r0 + dh : 1 + r0 + dh + ROWS_PER_TILE,
                             1 + dw : 1 + dw + W]
                    nc.tensor.matmul(ps[:], lhsT=lhsT.bitcast(MM_DTYPE),
                                     rhs=rhs.bitcast(MM_DTYPE),
                                     start=(ki == 0), stop=(ki == len(TAPS) - 1))
                nc.scalar.copy(stage[:, r0 : r0 + ROWS_PER_TILE, :], ps[:])
            nc.sync.dma_start(out_d[b, half * 128 : half * 128 + 128], stage[:])


_PROGRAM = None


def kernel(x, convs, w1, b1, w2, b2):
    global _PROGRAM, LAST_RESULTS
    x = np.ascontiguousarray(np.asarray(x, dtype=np.float32))
    convs = np.asarray(convs, dtype=np.float32)
    w1 = np.asarray(w1, dtype=np.float32)
    b1 = np.asarray(b1, dtype=np.float32)
    w2 = np.asarray(w2, dtype=np.float32)
    b2 = np.asarray(b2, dtype=np.float32)

    if _PROGRAM is None:
        _PROGRAM = _build_program()
    nc = _PROGRAM

    # host-side layout prep (dtype-preserving permutes only)
    convs_r = np.ascontiguousarray(convs.transpose(2, 0, 3, 4, 1)).reshape(CIN, E * TAPCO)
    w1t = np.ascontiguousarray(w1.T) / float(HWN)
    b1c = np.ascontiguousarray(b1[:, None])
    w2a = np.ascontiguousarray(np.concatenate([w2.T, b2[None, :]], axis=0))

    in_maps = [
        {
            "x": np.ascontiguousarray(x[c * BL : (c + 1) * BL]),
            "convs": convs_r,
            "w1t": w1t,
            "b1": b1c,
            "w2a": w2a,
        }
        for c in range(NCORES)
    ]
    res = run_bass_kernel_spmd(nc, in_maps, core_ids=list(range(NCORES)), trace=TRACE)
    LAST_RESULTS = res
    return np.concatenate([res.results[c]["out"] for c in range(NCORES)], axis=0)


# revision 43
# speedup vs baseline: 1.2650x; 1.2650x over previous
"""DyConv (MoE-routed dynamic convolution) Trainium2 Bass kernel.

Data-parallel over batch: 32 samples -> 8 cores x 4 samples.
Per sample, fully on-device:
  gap  = mean(x, HW)                  (VectorE reduce, scale folded into w1)
  h    = relu(gap @ w1.T + b1)        (TensorE matmul K=Cin=128, ScalarE relu)
  l    = h @ w2.T + b2                (TensorE matmul K=17 w/ ones-row bias trick)
  r    = softmax(l / 30)              (VectorE max, ScalarE exp+sum, VectorE recip)
  kern = sum_e r[e] * convs[e]        (VectorE scalar_tensor_tensor FMAs)
  out  = conv2d(x, kern, pad=1)       (9 shifted matmuls accumulated in PSUM,
                                       Cin=128 partition contraction, Cout=2x128,
                                       7 row tiles of N=8*56=448, float32r)
"""

import os
from contextlib import ExitStack

import numpy as np

import concourse.bass as bass
import concourse.bacc as bacc
import concourse.tile as tile
from concourse import mybir
from concourse.bass_utils import run_bass_kernel_spmd

F32 = mybir.dt.float32
F32R = mybir.dt.float32r

B, CIN, H, W = 32, 128, 56, 56
COUT, KS, E, R = 256, 3, 4, 16
NCORES = 8
BL = B // NCORES  # samples per core
TEMP = 30.0
HP, WP = H + 2, W + 2  # zero-padded image dims in SBUF
HWN = H * W  # 3136
ROWS_PER_TILE = 8
NTILES = H // ROWS_PER_TILE  # 7
NFREE = ROWS_PER_TILE * W  # 448 fp32 <= 512 (one PSUM bank)
TAPCO = KS * KS * COUT  # 2304, per-expert slice [tap, co]

# taps in kh-major order, matching the [ci, e, kh, kw, co] host layout
TAPS = [(dh, dw) for dh in (-1, 0, 1) for dw in (-1, 0, 1)]

# module-level knobs for test.py
TRACE = os.environ.get("DYCONV_TRACE", "0") == "1"
LAST_RESULTS = None
MM_DTYPE = F32R
# benchmarking: wrap the whole kernel body in a For_i loop of this many
# iterations (one NEFF, repeated device-side) so wall-clock timing is
# dominated by device time, not axon dispatch RTT.
LOOP_REPS = int(os.environ.get("DYCONV_LOOP_REPS", "1"))


def _build_program():
    # Bacc (not raw Bass): its compile() runs move_matmul_waits_to_ldweights
    # + generate_event_semaphores, legalizing instructions that need more
    # than one hardware sync-wait slot.
    nc = bacc.Bacc("TRN2", target_bir_lowering=False, debug=False)
    # x and convs feed float32r matmuls; host pre-rounds both to the fp32r
    # grid (RNE to 11 mantissa bits) so every on-chip conversion to f32r is
    # value-preserving. The on-chip f32r producers (DVE copy / mixing) are
    # what satisfies the BIR verifier's rounded-producer rule.
    x_d = nc.dram_tensor("x", [BL, CIN, H, W], F32, kind="ExternalInput").ap()
    # host-prearranged: convs_r[ci, e, kh, kw, co] flattened to [128, E*9*COUT]
    convs_d = nc.dram_tensor("convs", [CIN, E * TAPCO], F32R, kind="ExternalInput").ap()
    # w1.T / (H*W)  -> [CIN, R]
    w1t_d = nc.dram_tensor("w1t", [CIN, R], F32, kind="ExternalInput").ap()
    b1_d = nc.dram_tensor("b1", [R, 1], F32, kind="ExternalInput").ap()
    # w2.T -> [R, E]; g = exp(b2/TEMP) -> [1, E]: the bias enters softmax
    # as a constant per-expert multiplicative factor applied after exp.
    w2t_d = nc.dram_tensor("w2t", [R, E], F32, kind="ExternalInput").ap()
    g_d = nc.dram_tensor("g", [1, E], F32, kind="ExternalInput").ap()
    out_d = nc.dram_tensor("out", [BL, COUT, H, W], F32, kind="ExternalOutput").ap()

    with tile.TileContext(nc) as tc, ExitStack() as ctx:
        if LOOP_REPS > 1:
            with tc.For_i(0, LOOP_REPS, 1, hint_engines=(mybir.EngineType.PE,)):
                _emit(ctx, tc, x_d, convs_d, w1t_d, b1_d, w2t_d, g_d, out_d)
        else:
            _emit(ctx, tc, x_d, convs_d, w1t_d, b1_d, w2t_d, g_d, out_d)
    nc.compile()
    return nc


def _emit(ctx, tc, x_d, convs_d, w1t_d, b1_d, w2t_d, g_d, out_d):
    # The fp32/fp32r matmul hardware encoding (fused 4-byte weight load,
    # S3_LW) carries at most ONE sync wait, so every matmul is arranged to
    # depend on a single engine's semaphore:
    #   conv matmuls  -> DVE only (kern mixing, padded-image copy, PSUM
    #                    bank release via DVE stage copies)
    #   router mm1/mm2-> ACT only (gap via ACT accumulate, relu on ACT)
    #   rb broadcast  -> DVE only
    # One-time DMA waits for the router weights are absorbed by warmup
    # matmuls that read only those tiles.
    nc = tc.nc

    const_pool = ctx.enter_context(tc.tile_pool(name="const", bufs=1))
    xp_pool = ctx.enter_context(tc.tile_pool(name="xpad", bufs=3))
    kern_pool = ctx.enter_context(tc.tile_pool(name="kern", bufs=2))
    small_pool = ctx.enter_context(tc.tile_pool(name="small", bufs=2))
    stage_pool = ctx.enter_context(tc.tile_pool(name="stage", bufs=4))
    psum_pool = ctx.enter_context(tc.tile_pool(name="psum", bufs=3, space="PSUM"))
    psum_r_pool = ctx.enter_context(tc.tile_pool(name="psum_r", bufs=1, space="PSUM"))

    # resident weights (convs is DMA'd in 4 per-expert chunks, emitted
    # after sample 0's x DMA so the first image load isn't queued behind it)
    convs_sb = const_pool.tile([CIN, E * TAPCO], F32R)
    w1t_sb = const_pool.tile([CIN, R], F32)
    nc.sync.dma_start(w1t_sb[:], w1t_d[:])
    b1_sb = const_pool.tile([R, 1], F32)
    nc.sync.dma_start(b1_sb[:], b1_d[:])
    w2t_sb = const_pool.tile([R, E], F32)
    nc.sync.dma_start(w2t_sb[:], w2t_d[:])
    g_sb = const_pool.tile([1, E], F32)
    nc.sync.dma_start(g_sb[:], g_d[:])
    ones_sb = const_pool.tile([1, CIN], F32)
    nc.vector.memset(ones_sb[:], 1.0)
    # ACT writes its copy-with-accumulate output here; only the accumulator
    # (the GAP vector) is consumed.
    gap_scratch = const_pool.tile([CIN, H * W], F32)

    # warmup matmuls: absorb the router-weight DMA waits into PE's clock
    # so the per-sample router matmuls need only their ACT dependency.
    warm1 = psum_r_pool.tile([R, R], F32, tag="warm1")
    nc.tensor.matmul(warm1[:], lhsT=w1t_sb[:, 0:R], rhs=w1t_sb[:, 0:R],
                     start=True, stop=True)
    warm2 = psum_r_pool.tile([E, E], F32, tag="warm2")
    nc.tensor.matmul(warm2[:], lhsT=w2t_sb[:, 0:E], rhs=w2t_sb[:, 0:E],
                     start=True, stop=True)

    # two persistent zero-padded fp32r images, borders zeroed once via DVE
    # copies from an f32 zero row (memset cannot target f32r directly, and
    # the f32->f32r copy is a legal "rounded" producer for the matmuls)
    zrow = const_pool.tile([CIN, WP], F32)
    nc.vector.memset(zrow[:], 0.0)
    xprs = []
    for i in range(2):
        t = const_pool.tile([CIN, HP, WP], F32R, tag=f"xpr{i}")
        nc.vector.tensor_copy(t[:, 0, :], zrow[:])
        nc.vector.tensor_copy(t[:, HP - 1, :], zrow[:])
        nc.vector.tensor_copy(t[:, 1 : HP - 1, 0], zrow[:, 0 : HP - 2])
        nc.vector.tensor_copy(t[:, 1 : HP - 1, WP - 1], zrow[:, 0 : HP - 2])
        xprs.append(t)

    def prep(b):
        """Per-sample producer work (DMA/ACT/DVE + tiny router matmuls):
        image load, GAP, padded-image copy, router, expert mixing."""
        xp = xp_pool.tile([CIN, H, W], F32)
        nc.sync.dma_start(xp[:], x_d[b])
        if b == 0:
            for e in range(E):
                nc.sync.dma_start(convs_sb[:, e * TAPCO : (e + 1) * TAPCO],
                                  convs_d[:, e * TAPCO : (e + 1) * TAPCO])

        # GAP on DVE (keeps ACT free for the PSUM stage copies)
        gap = small_pool.tile([CIN, 1], F32, tag="gap")
        nc.vector.reduce_sum(gap[:], xp[:], axis=mybir.AxisListType.XY)

        # materialize padded fp32r image through DVE (f32 -> f32r copy)
        xpr = xprs[b % 2]
        nc.vector.tensor_copy(xpr[:, 1 : H + 1, 1 : W + 1], xp[:])

        # router (PE deps: ACT only)
        ph = psum_r_pool.tile([R, 1], F32, tag="ph")
        nc.tensor.matmul(ph[:], lhsT=w1t_sb[:], rhs=gap[:], start=True, stop=True)
        hmid = small_pool.tile([R, 1], F32, tag="hmid")
        nc.scalar.activation(hmid[:], ph[:], mybir.ActivationFunctionType.Relu,
                             bias=b1_sb[:], scale=1.0)
        pl = psum_r_pool.tile([1, E], F32, tag="pl")
        nc.tensor.matmul(pl[:], lhsT=hmid[:], rhs=w2t_sb[:],
                         start=True, stop=True)

        # softmax(logits/TEMP + b2/TEMP): logits here are O(0.1) by
        # construction (router weights ~0.05, gap ~N(0, 1/sqrt(HW))), so no
        # max-shift is needed; b2 enters as the constant factor exp(b2/TEMP).
        ex = small_pool.tile([1, E], F32, tag="ex")
        nc.scalar.activation(ex[:], pl[:], mybir.ActivationFunctionType.Exp,
                             scale=1.0 / TEMP)
        exg = small_pool.tile([1, E], F32, tag="exg")
        nc.vector.tensor_mul(exg[:], ex[:], g_sb[:])
        ssum = small_pool.tile([1, 1], F32, tag="ssum")
        nc.vector.reduce_sum(ssum[:], exg[:], axis=mybir.AxisListType.X)
        rec = small_pool.tile([1, 1], F32, tag="rec")
        nc.vector.reciprocal(rec[:], ssum[:])
        rt = small_pool.tile([1, E], F32, tag="rt")
        nc.vector.tensor_scalar_mul(rt[:], exg[:], rec[:])
        # broadcast routing weights to all 128 partitions: ones[1,128].T @ rt
        rb = psum_r_pool.tile([CIN, E], F32, tag="rb")
        nc.tensor.matmul(rb[:], lhsT=ones_sb[:], rhs=rt[:], start=True, stop=True)

        # mix expert kernels: kern[ci, tap, co] = sum_e r[e]*convs[ci,e,tap,co]
        # kern is float32r (rounded on DVE write) so the conv matmuls see a
        # rounded producer; DVE reads use f32 bitcasts of the same bits.
        kern = kern_pool.tile([CIN, TAPCO], F32R)
        nc.vector.tensor_scalar_mul(kern[:], convs_sb[:, 0:TAPCO].bitcast(F32),
                                    rb[:, 0:1])
        for e in range(1, E):
            nc.vector.scalar_tensor_tensor(
                kern[:], convs_sb[:, e * TAPCO : (e + 1) * TAPCO].bitcast(F32),
                rb[:, e : e + 1], kern[:].bitcast(F32),
                op0=mybir.AluOpType.mult, op1=mybir.AluOpType.add)
        return xpr, kern

    def conv_half(b, half, xpr, kern):
        """One Cout-half of the conv: 7 row tiles x 9 taps accumulated in
        PSUM, staged to SBUF on ACT, then DMA'd out."""
        stage = stage_pool.tile([128, H, W], F32)
        for t in range(NTILES):
            ps = psum_pool.tile([128, ROWS_PER_TILE, W], F32)
            r0 = ROWS_PER_TILE * t
            for ki, (dh, dw) in enumerate(TAPS):
                lhsT = kern[:, ki * COUT + half * 128 : ki * COUT + half * 128 + 128]
                rhs = xpr[:, 1 + r0 + dh : 1 + r0 + dh + ROWS_PER_TILE,
                          1 + dw : 1 + dw + W]
                nc.tensor.matmul(ps[:], lhsT=lhsT, rhs=rhs,
                                 start=(ki == 0), stop=(ki == len(TAPS) - 1))
            nc.scalar.copy(stage[:, r0 : r0 + ROWS_PER_TILE, :], ps[:])
        nc.sync.dma_start(out_d[b, half * 128 : half * 128 + 128], stage[:])

    # software pipeline: emit sample b+1's producer work between sample b's
    # two conv halves so the mixing for b+1 overlaps b's matmuls instead of
    # serializing behind them (engines execute their streams in order).
    state = prep(0)
    for b in range(BL):
        xpr, kern = state
        conv_half(b, 0, xpr, kern)
        if b + 1 < BL:
            state = prep(b + 1)
        conv_half(b, 1, xpr, kern)


_PROGRAM = None


def round_fp32r(a: np.ndarray) -> np.ndarray:
    """Round fp32 to the fp32r grid: RNE to 11 mantissa bits (top 20 bits
    kept, low 12 zero) — matches walrus's fp32_to_fp32r/fp32r_to_fp32."""
    u = np.ascontiguousarray(a, dtype=np.float32).view(np.uint32)
    drop = 12
    lsb = (u >> drop) & np.uint32(1)
    r = u + (np.uint32((1 << (drop - 1)) - 1) + lsb)
    r &= np.uint32(0xFFFFFFFF) ^ np.uint32((1 << drop) - 1)
    return r.view(np.float32)


def kernel(x, convs, w1, b1, w2, b2):
    global _PROGRAM, LAST_RESULTS
    x = np.ascontiguousarray(np.asarray(x, dtype=np.float32))
    convs = np.asarray(convs, dtype=np.float32)
    w1 = np.asarray(w1, dtype=np.float32)
    b1 = np.asarray(b1, dtype=np.float32)
    w2 = np.asarray(w2, dtype=np.float32)
    b2 = np.asarray(b2, dtype=np.float32)

    if _PROGRAM is None:
        _PROGRAM = _build_program()
    nc = _PROGRAM

    # host-side layout prep (permutes + fp32r grid rounding)
    x = round_fp32r(x)
    convs_r = round_fp32r(
        np.ascontiguousarray(convs.transpose(2, 0, 3, 4, 1)).reshape(CIN, E * TAPCO))
    w1t = np.ascontiguousarray(w1.T) / float(HWN)
    b1c = np.ascontiguousarray(b1[:, None])
    w2t = np.ascontiguousarray(w2.T)
    g = np.ascontiguousarray(np.exp(b2 / TEMP)[None, :]).astype(np.float32)

    in_maps = [
        {
            "x": np.ascontiguousarray(x[c * BL : (c + 1) * BL]),
            "convs": convs_r,
            "w1t": w1t,
            "b1": b1c,
            "w2t": w2t,
            "g": g,
        }
        for c in range(NCORES)
    ]
    res = run_bass_kernel_spmd(nc, in_maps, core_ids=list(range(NCORES)), trace=TRACE)
    LAST_RESULTS = res
    return np.concatenate([res.results[c]["out"] for c in range(NCORES)], axis=0)
